# revision 9
# baseline (speedup 1.0000x reference)
# MoE top-2 routing kernel for 8 Trainium2 NeuronCores (expert-parallel).
#
# Problem (hardcoded shapes): T=2048 tokens, D=2048 model dim, F=4096 ffn dim,
# E=8 experts, top-2 routing with renormalized softmax weights.
#
# Sharding: one expert per core. The host does dispatch/data placement: an
# fp32 router pre-pass picks each token's top-2 experts (selection is
# numerically unambiguous: min 2nd-vs-3rd logit gap is ~7e-4 for these
# inputs, far above fp32 matmul noise), computes the renormalized top-2
# softmax weights in float64, gathers each expert's tokens into a transposed
# capacity buffer xT_e [D, C] (C = max expert load, NOT rounded to 128), and
# zero-pads the tail. Padded columns are harmless: MLP(0) = 0 and their
# router weight is set to 0.
#
# Device layout is fully weight-stationary, tokens always moving in columns:
#   g[f,t] = sum_d wg[d,f] x[d,t]   (lhsT = 128x128 wg tile, rhs = xT cols)
#   u[f,t] likewise; h[f,t] = silu(g)*u lands directly in [f,t] layout, so
#   the down matmul y[d,t] = sum_f wd[f,d] h[f,t] needs NO PE transposes
#   (the previous x-stationary design burned ~44us in 128x128 transposes and
#   padded tokens to a multiple of 128). The per-token router weight is a
#   host-provided [128, C] broadcast and is applied for free inside the
#   PSUM->SBUF copy of y. Output is yT [D, C]; the host scatter-adds its
#   transpose into [T, D] (each token lives on exactly its 2 routed cores).
#
# Tokens stream in PSUM-bank-sized column chunks (<=512 fp32); weights are
# host-retiled so every weight DMA is one [128, D|F] contiguous block.
# PE work per core: 3 * 512 weight tiles * C columns ~= 1536*536 cycles
# ~= 343us at 2.4 GHz bf16 (1 col/cycle), vs 565us for the baseline.

import os
import numpy as np
import ml_dtypes

_BF16NP = ml_dtypes.bfloat16

import concourse.bass as bass
import concourse.bacc as bacc
import concourse.mybir as mybir
import concourse.tile as tile
from concourse import bass_utils

FP32 = mybir.dt.float32
BF16 = mybir.dt.bfloat16
ACTF = mybir.ActivationFunctionType

T, D, F, E = 2048, 2048, 4096, 8
NCORES = 8
ND = D // 128    # 16 d-tiles
NF = F // 128    # 32 f-tiles


def _chunks_for(C):
    """Split C token columns into PSUM-bank-sized chunks (<=512 fp32 cols)."""
    nch = (C + 511) // 512
    out, rem, c0 = [], C, 0
    for i in range(nch):
        cn = -(-(rem // (nch - i)) // 4) * 4
        cn = min(cn, rem)
        out.append((c0, cn))
        c0 += cn
        rem -= cn
    return out


def build_program(C):
    chunks = _chunks_for(C)
    nc = bacc.Bacc(
        "TRN2",
        target_bir_lowering=False,
        debug=False,
        enable_asserts=False,
        num_devices=NCORES,
    )
    # x in [p, d, t] tile layout [128, 16*C]: row p, col d*C+t holds
    # xT[d*128+p, t]; DMA'd in 4 groups of 4 d-tiles (4.3KB/partition rows)
    x_d = nc.dram_tensor("x", [128, ND * C], BF16, kind="ExternalInput").ap()
    # router weight per token, broadcast to [128, C] on host, fp32
    wb_d = nc.dram_tensor("wb", [128, C], FP32, kind="ExternalInput").ap()
    # retiled weights: wg/wu rows fi*128+p, cols d*128+q  (= wg[d*128+p, fi*128+q])
    wg_d = nc.dram_tensor("wg", [F, D], BF16, kind="ExternalInput").ap()
    wu_d = nc.dram_tensor("wu", [F, D], BF16, kind="ExternalInput").ap()
    # retiled wd: rows dt*128+p, cols fi*128+q  (= wd[fi*128+p, dt*128+q])
    wd_d = nc.dram_tensor("wd", [D, F], BF16, kind="ExternalInput").ap()
    # output yT [D, C] fp32
    y_d = nc.dram_tensor("y", [D, C], FP32, kind="ExternalOutput").ap()

    with tile.TileContext(nc) as tc:
        with (
            tc.tile_pool(name="const", bufs=1) as const_pool,
            tc.tile_pool(name="xp", bufs=1) as x_pool,
            tc.tile_pool(name="hp", bufs=1) as h_pool,
            tc.tile_pool(name="wgu", bufs=6) as wgu_pool,
            tc.tile_pool(name="wdp", bufs=3) as wd_pool,
            tc.tile_pool(name="yp", bufs=4) as y_pool,
            tc.tile_pool(name="stp", bufs=4) as st_pool,
            tc.tile_pool(name="ps", bufs=8, space="PSUM") as ps_pool,
        ):
            # ---- PE warmup: ~5us of throwaway matmuls on scratch data so
            # the HAM clock-gate opens to 8/8 while the startup DMAs land,
            # and the real MM stream starts warm. No data dependencies. ----
            dum = const_pool.tile([128, 160], BF16, tag="dum", name="dum")
            nc.vector.memset(dum[:], 0.0)
            pdum = ps_pool.tile([128, 512], FP32, tag="ps", name="ps")
            for _ in range(42):
                nc.tensor.matmul(pdum[:, :160], dum[:, :128], dum[:],
                                 start=True, stop=True)

            # startup-critical DMAs: weights on the sync HWDGE ring, x/wb
            # (and later y stores) on the scalar HWDGE ring so the two
            # streams run in parallel (each ring is FIFO).
            wgt0 = wgu_pool.tile([128, D], BF16, tag="w", name="wgt")
            nc.sync.dma_start(wgt0[:], wg_d[0:128, :])
            xt = []
            for g in range(ND // 4):
                xg = x_pool.tile([128, 4 * C], BF16, tag=f"x{g}", name=f"x{g}")
                nc.scalar.dma_start(xg[:], x_d[:, g * 4 * C:(g + 1) * 4 * C])
                xt.append(xg)
            wut0 = wgu_pool.tile([128, D], BF16, tag="w", name="wut")
            nc.sync.dma_start(wut0[:], wu_d[0:128, :])
            wb_sb = const_pool.tile([128, C], FP32, tag="wb", name="wb_sb")
            nc.scalar.dma_start(wb_sb[:], wb_d[:])

            def xs(d, c0, cn):
                return xt[d // 4][:, (d % 4) * C + c0:(d % 4) * C + c0 + cn]

            # ---- phase 1: gate/up matmuls + silu*up -> h[f, t] ----
            hs = []
            for fi in range(NF):
                if fi == 0:
                    wgt, wut = wgt0, wut0
                else:
                    wgt = wgu_pool.tile([128, D], BF16, tag="w", name="wgt")
                    nc.sync.dma_start(wgt[:], wg_d[fi * 128:(fi + 1) * 128, :])
                    wut = wgu_pool.tile([128, D], BF16, tag="w", name="wut")
                    nc.sync.dma_start(wut[:], wu_d[fi * 128:(fi + 1) * 128, :])
                pg = [ps_pool.tile([128, 512], FP32, tag="ps", name="ps")
                      for _ in chunks]
                pu = [ps_pool.tile([128, 512], FP32, tag="ps", name="ps")
                      for _ in chunks]
                for d in range(ND):
                    lw = wgt[:, d * 128:(d + 1) * 128]
                    for ci, (c0, cn) in enumerate(chunks):
                        nc.tensor.matmul(
                            pg[ci][:, :cn], lw, xs(d, c0, cn),
                            start=(d == 0), stop=(d == ND - 1),
                        )
                for d in range(ND):
                    lw = wut[:, d * 128:(d + 1) * 128]
                    for ci, (c0, cn) in enumerate(chunks):
                        nc.tensor.matmul(
                            pu[ci][:, :cn], lw, xs(d, c0, cn),
                            start=(d == 0), stop=(d == ND - 1),
                        )
                h = h_pool.tile([128, C], BF16, tag=f"h{fi}", name=f"h{fi}")
                for ci, (c0, cn) in enumerate(chunks):
                    st = st_pool.tile([128, 512], FP32, tag="st", name="st")
                    nc.scalar.activation(st[:, :cn], pg[ci][:, :cn], ACTF.Silu)
                    nc.vector.tensor_mul(h[:, c0:c0 + cn], st[:, :cn],
                                         pu[ci][:, :cn])
                hs.append(h)

            # ---- phase 2: down matmuls, router-weight scale, store yT ----
            for dt in range(ND):
                wdt = wd_pool.tile([128, F], BF16, tag="wd", name="wdt")
                nc.sync.dma_start(wdt[:], wd_d[dt * 128:(dt + 1) * 128, :])
                # chunk-outer: chunk 0's scale+store overlaps chunk 1's MMs,
                # so only the last chunk's store is exposed at the tail
                for ci, (c0, cn) in enumerate(chunks):
                    py = ps_pool.tile([128, 512], FP32, tag="ps", name="ps")
                    for fi in range(NF):
                        nc.tensor.matmul(
                            py[:, :cn], wdt[:, fi * 128:(fi + 1) * 128],
                            hs[fi][:, c0:c0 + cn],
                            start=(fi == 0), stop=(fi == NF - 1),
                        )
                    ysb = y_pool.tile([128, 512], FP32, tag="y", name="ysb")
                    nc.vector.tensor_mul(ysb[:, :cn], py[:, :cn],
                                         wb_sb[:, c0:c0 + cn])
                    nc.scalar.dma_start(
                        y_d[dt * 128:(dt + 1) * 128, c0:c0 + cn], ysb[:, :cn])

    nc.compile()
    return nc


_PROGRAM_CACHE = {}


def _get_program(C):
    if C not in _PROGRAM_CACHE:
        _PROGRAM_CACHE[C] = build_program(C)
    return _PROGRAM_CACHE[C]


def _route_host(x_TD, router_w):
    """Host dispatch: top-2 ids + renormalized top-2 softmax weights."""
    logits = (x_TD @ router_w).astype(np.float64)  # selection gap >> fp32 err
    order = np.argsort(-logits, axis=1, kind="stable")
    top2 = order[:, :2]
    z = logits - logits.max(axis=1, keepdims=True)
    p = np.exp(z)
    p /= p.sum(axis=1, keepdims=True)
    pw = np.take_along_axis(p, top2, axis=1)       # [T, 2]
    pw /= pw.sum(axis=1, keepdims=True)
    return top2, pw


def _retile_wgu(w):
    """[D, F] -> [F, D] with rows fi*128+p, cols d*128+q, bf16."""
    m = w.astype(_BF16NP).reshape(ND, 128, NF, 128).transpose(2, 1, 0, 3)
    return np.ascontiguousarray(m).reshape(F, D)


def _retile_wd(w):
    """[F, D] -> [D, F] with rows dt*128+p, cols fi*128+q, bf16."""
    m = w.astype(_BF16NP).reshape(NF, 128, ND, 128).transpose(2, 1, 0, 3)
    return np.ascontiguousarray(m).reshape(D, F)


def kernel_with_results(x_TD, router_w, w_gate, w_up, w_down):
    x_TD = np.ascontiguousarray(x_TD, np.float32)
    router_w = np.ascontiguousarray(router_w, np.float32)
    w_gate = np.ascontiguousarray(w_gate, np.float32)
    w_up = np.ascontiguousarray(w_up, np.float32)
    w_down = np.ascontiguousarray(w_down, np.float32)

    top2, pw = _route_host(x_TD, router_w)
    idx_lists = [np.where((top2 == e).any(axis=1))[0] for e in range(E)]
    max_cnt = max(len(ix) for ix in idx_lists)
    C = max(64, -(-max_cnt // 8) * 8)

    nc = _get_program(C)

    in_maps = []
    for e in range(E):
        ix = idx_lists[e]
        n = len(ix)
        xg = np.zeros((C, D), np.float32)
        xg[:n] = x_TD[ix]
        # [p, d, t] tile layout, contiguous per (p, d-group) for fat DMA rows
        xTe = np.ascontiguousarray(
            xg.T.astype(_BF16NP).reshape(ND, 128, C).transpose(1, 0, 2)
        ).reshape(128, ND * C)
        # this expert's renormalized weight for each of its tokens
        sel = (top2[ix] == e).argmax(axis=1)
        wtok = np.zeros((C,), np.float32)
        wtok[:n] = pw[ix, sel]
        wb = np.ascontiguousarray(
            np.broadcast_to(wtok[None, :], (128, C)), np.float32)
        in_maps.append({
            "x": xTe,
            "wb": wb,
            "wg": _retile_wgu(w_gate[e]),
            "wu": _retile_wgu(w_up[e]),
            "wd": _retile_wd(w_down[e]),
        })

    try:
        res = bass_utils.run_bass_kernel_spmd(
            nc, in_maps, core_ids=list(range(NCORES))
        )
    except ModuleNotFoundError:
        # Tracing requested via env but the axon NTFF hook module is absent
        # in this image — rerun without tracing.
        os.environ["BASS_NEVER_TRACE"] = "1"
        res = bass_utils.run_bass_kernel_spmd(
            nc, in_maps, core_ids=list(range(NCORES))
        )

    out = np.zeros((T, D), np.float32)
    for e in range(E):
        ix = idx_lists[e]
        y = res.results[e]["y"]  # [D, C]
        out[ix] += y[:, :len(ix)].T
    return out, res


def kernel(**inputs):
    out, _ = kernel_with_results(**inputs)
    return out
